# revision 11
# baseline (speedup 1.0000x reference)
"""Trainium2 Bass kernel for causal self-attention (B=4, T=2048, C=1024, H=16).

Sharding: 8 cores = 4 batch-pairs x 2-way tensor parallel over heads.
Core c handles batch c//2 and heads [8*(c%2), 8*(c%2)+8).

v2 (bf16 rework of the 697us baseline):
  - all matmul operands bf16 (host converts x/W to bf16; halves DMA + SBUF)
  - v projection at N=512 (V columns only); the per-head ones column of the
    [V|1] attention lhsT is written once by a strided memset
  - S^T pair per head-pair via tile_position row groups (0,0)/(64,0) --
    measured concurrent on HW (2 matmuls ~ cost of one)
  - exp over a merged [128, 2048] strip (both heads) in ONE ACT instruction:
    80 exp instrs instead of 224 (ACT was ~60% busy in the model)
  - outproj PSUM->SBUF copies moved from ACT to DVE; output DMA'd as bf16,
    host sums the two TP partials in f32 and adds b_proj
"""

import os
import sys

sys.path.insert(0, "/opt/trn_rl_repo")

import numpy as np

import concourse.bass as bass
import concourse.tile as tile
from concourse import bacc, mybir
from concourse.bass_utils import run_bass_kernel_spmd

F32 = mybir.dt.float32
BF16 = mybir.dt.bfloat16
AF = mybir.ActivationFunctionType
NPBF16 = mybir.dt.np(mybir.dt.bfloat16)

B, T, C, H, HD = 4, 2048, 1024, 16, 64
N_CORES = 8
HL = 8          # heads per core
VW = HL * (HD + 1)  # 520: v strip width (64 cols + ones col per head)

LAST_RESULT = None  # BassKernelResults of the most recent run (for test.py)
_CACHED = None      # (nc,) build cache


def build_kernel(loops=1):
    nc = bacc.Bacc(
        "TRN2",
        target_bir_lowering=False,
        debug=False,
        enable_asserts=False,
        num_devices=N_CORES,
    )
    d_xT = nc.dram_tensor("xT", [C, T], BF16, kind="ExternalInput").ap()
    d_wqk = nc.dram_tensor("wqk", [C, C], BF16, kind="ExternalInput").ap()
    d_wv = nc.dram_tensor("wv", [C, HL * HD], BF16, kind="ExternalInput").ap()
    d_bqk = nc.dram_tensor("bqk", [C], F32, kind="ExternalInput").ap()
    d_bv = nc.dram_tensor("bv", [HL * HD], F32, kind="ExternalInput").ap()
    d_wp = nc.dram_tensor("wp", [HL * HD, C], BF16, kind="ExternalInput").ap()
    d_mask = nc.dram_tensor("mask", [128, 128], BF16, kind="ExternalInput").ap()
    d_ident = nc.dram_tensor("ident", [128, 128], BF16, kind="ExternalInput").ap()
    d_out = nc.dram_tensor("out", [T, C], BF16, kind="ExternalOutput").ap()

    with tile.TileContext(nc) as tc:
        for _ in range(loops):
            kernel_body(tc, d_xT, d_wqk, d_wv, d_bqk, d_bv, d_wp, d_mask, d_ident,
                        d_out)
    nc.compile()
    return nc


def kernel_body(tc, d_xT, d_wqk, d_wv, d_bqk, d_bv, d_wp, d_mask, d_ident, d_out):
    nc = tc.nc
    from contextlib import ExitStack

    ctx = ExitStack()
    with ctx:
        # ---- pools (stack allocator: persistent first) ----
        p_misc = ctx.enter_context(tc.tile_pool(name="misc", bufs=1))
        p_kT = ctx.enter_context(tc.tile_pool(name="kT", bufs=1))
        p_v = ctx.enter_context(tc.tile_pool(name="v", bufs=1))
        p_w1 = ctx.enter_context(tc.tile_pool(name="w1", bufs=1))
        p_q = ctx.enter_context(tc.tile_pool(name="q", bufs=2))
        p_yT = ctx.enter_context(tc.tile_pool(name="yT", bufs=2))
        p_xt = ctx.enter_context(tc.tile_pool(name="xt", bufs=1))
        p_pt = ctx.enter_context(tc.tile_pool(name="pt", bufs=3))
        p_sm = ctx.enter_context(tc.tile_pool(name="sm", bufs=1))
        p_po = ctx.enter_context(tc.tile_pool(name="po", bufs=2))
        p_ps = ctx.enter_context(tc.tile_pool(name="ps", bufs=2, space="PSUM"))
        p_ps_s = ctx.enter_context(tc.tile_pool(name="ps_s", bufs=2, space="PSUM"))
        p_ps_o = ctx.enter_context(tc.tile_pool(name="ps_o", bufs=1, space="PSUM"))

        mask_s = p_misc.tile([128, 128], BF16)
        nc.sync.dma_start(out=mask_s, in_=d_mask)
        ident_s = p_misc.tile([128, 128], BF16)
        nc.sync.dma_start(out=ident_s, in_=d_ident)
        bqk_s = p_misc.tile([128, 8], F32)
        nc.sync.dma_start(out=bqk_s, in_=d_bqk.rearrange("(a p) -> p a", p=128))
        bv_s = p_misc.tile([1, 512], F32)
        nc.sync.dma_start(out=bv_s, in_=d_bv.rearrange("(o a) -> o a", o=1))
        bvb_s = p_misc.tile([128, 512], F32)
        nc.gpsimd.partition_broadcast(bvb_s, bv_s, channels=128)

        kT_s = p_kT.tile([128, 4, T], BF16)     # K chunks: heads (2j, 2j+1)
        v_s = p_v.tile([128, 16, VW], BF16)     # t-tile r -> [V|1] rows
        wqk_s = p_w1.tile([128, 8, C], BF16)
        wv_s = p_w1.tile([128, 8, 512], BF16)
        wp_s = p_w1.tile([128, 4, C], BF16)

        # per-head ones columns of the [V|1] lhsT, written once
        nc.gpsimd.memset(
            v_s.rearrange("p t (h c) -> p t h c", h=HL)[:, :, :, HD:HD + 1], 1.0
        )

        q_tiles = {}
        yT_tiles = {}

        def proj_items(tt):
            xt_s = p_xt.tile([128, 8, 512], BF16, tag="xt", name="xt_s")
            if tt == 0:
                # wqk+xt are the first qk-group's critical path; wv is only
                # needed by the V groups, so it queues after them
                for i in range(8):
                    nc.sync.dma_start(
                        out=wqk_s[:, i, :],
                        in_=d_wqk[128 * i:128 * i + 128, :],
                    )
                    nc.sync.dma_start(
                        out=xt_s[:, i, :],
                        in_=d_xT[128 * i:128 * i + 128, 0:512],
                    )
                for i in range(8):
                    nc.sync.dma_start(
                        out=wv_s[:, i, :],
                        in_=d_wv[128 * i:128 * i + 128, :],
                    )
                for i in range(4):
                    nc.sync.dma_start(
                        out=wp_s[:, i, :],
                        in_=d_wp[128 * i:128 * i + 128, :],
                    )
            else:
                for i in range(8):
                    nc.sync.dma_start(
                        out=xt_s[:, i, :],
                        in_=d_xT[128 * i:128 * i + 128, 512 * tt:512 * tt + 512],
                    )
            q_s = p_q.tile([128, 4, 512], BF16, tag="q", name="q_s")
            q_tiles[tt] = q_s

            def qk_group(j):
                ps = p_ps.tile([128, 512], F32, tag="p1", name="ps_p1")
                for i in range(8):
                    nc.tensor.matmul(
                        ps,
                        lhsT=wqk_s[:, i, 128 * j:128 * j + 128],
                        rhs=xt_s[:, i, :],
                        start=(i == 0),
                        stop=(i == 7),
                    )
                dest = (
                    q_s[:, j, :] if j < 4
                    else kT_s[:, j - 4, 512 * tt:512 * tt + 512]
                )
                nc.vector.tensor_scalar_add(out=dest, in0=ps, scalar1=bqk_s[:, j:j + 1])

            def v_group(st):
                ts_ = 4 * tt + st
                psv = p_ps.tile([128, 512], F32, tag="p1", name="ps_v")
                for i in range(8):
                    nc.tensor.matmul(
                        psv,
                        lhsT=xt_s[:, i, 128 * st:128 * st + 128],
                        rhs=wv_s[:, i, :],
                        start=(i == 0),
                        stop=(i == 7),
                    )
                nc.vector.tensor_add(
                    out=v_s[:, ts_, :].rearrange("p (h c) -> p h c", c=HD + 1)[:, :, 0:HD],
                    in0=psv.rearrange("p (h c) -> p h c", c=HD),
                    in1=bvb_s.rearrange("p (h c) -> p h c", c=HD),
                )

            # K chunk then Q chunk per pair: unblocks attention earliest
            qk_items = []
            for p4 in range(4):
                qk_items.append(lambda j=4 + p4: qk_group(j))
                qk_items.append(lambda j=p4: qk_group(j))
            v_items = [lambda st=st: v_group(st) for st in range(4)]
            return qk_items, v_items

        def attn_items(qt):
            """Returns list of (callable, req_qk, req_v): req_qk/req_v are how many
            of this slot's qk/v groups must be emitted before this item.

            PE-stream software pipeline: item i emits S^T(kr_i) then O^T(kr_{i-1}),
            so the O^T that waits on exp(kr_{i-1}) sits behind already-runnable
            S^T matmuls and never head-of-line blocks the next strip."""
            q_s = q_tiles[qt]
            yT_b = p_yT.tile([128, 4, 512], BF16, tag="yT", name="yT_b")
            yT_tiles[qt] = yT_b
            items = []
            o_tiles = {}
            pt_tiles = {}

            def s_group(p4, kr):
                if kr == 0:
                    o_tiles[p4] = {
                        hh: p_ps_o.tile([65, 512], F32, tag=f"o{hh}", name=f"o{hh}")
                        for hh in range(2)
                    }
                strip = p_ps_s.tile([128, 1024], F32, tag="s", name="strip")
                p = kr - 4 * qt
                n0 = 128 * p if p > 0 else 0
                diag = p >= 0
                for hh in range(2):
                    poff = 64 * hh
                    nc.tensor.matmul(
                        strip[:, 512 * hh + n0:512 * hh + 512],
                        lhsT=kT_s[poff:poff + 64, p4, 128 * kr:128 * kr + 128],
                        rhs=q_s[poff:poff + 64, p4, n0:512],
                        start=True,
                        stop=not diag,
                        tile_position=(poff, 0),
                        skip_group_check=diag,
                    )
                    if diag:
                        # accumulate -30000 into the above-diagonal half of the
                        # boundary block (ident.T @ maskA = maskA) so exp -> 0;
                        # replaces a gpsimd mask multiply on the exp->O^T path
                        nc.tensor.matmul(
                            strip[:, 512 * hh + n0:512 * hh + n0 + 128],
                            lhsT=ident_s,
                            rhs=mask_s,
                            start=False,
                            stop=True,
                            skip_group_check=True,
                        )
                pt_t = p_pt.tile([128, 1024], BF16, tag="pt", name="pt_t")
                pt_tiles[(p4, kr)] = pt_t
                nc.scalar.activation(out=pt_t, in_=strip, func=AF.Exp, scale=0.125)

            def o_group(p4, kr):
                o_ps = o_tiles[p4]
                pt_t = pt_tiles.pop((p4, kr))
                p = kr - 4 * qt
                n0 = 128 * p if p > 0 else 0
                for hh in range(2):
                    hl = 2 * p4 + hh
                    nc.tensor.matmul(
                        o_ps[hh][0:65, n0:512],
                        lhsT=v_s[:, kr, 65 * hl:65 * hl + 65],
                        rhs=pt_t[:, 512 * hh + n0:512 * hh + 512],
                        start=(kr == 0),
                        stop=(kr == 4 * qt + 3),
                    )

            def norm(p4):
                o_ps = o_tiles[p4]
                for hh in range(2):
                    recl = p_sm.tile([1, 512], F32, tag="recl", name="recl")
                    nc.vector.reciprocal(out=recl, in_=o_ps[hh][64:65, :])
                    lb_s = p_sm.tile([64, 512], F32, tag="lbs", name="lb_s")
                    nc.gpsimd.partition_broadcast(lb_s, recl, channels=64)
                    if hh == 0:
                        nc.vector.tensor_mul(
                            out=yT_b[0:64, p4, :], in0=o_ps[hh][0:64, :], in1=lb_s
                        )
                    else:
                        tmp = p_sm.tile([64, 512], BF16, tag="tmp", name="tmp")
                        nc.vector.tensor_mul(out=tmp, in0=o_ps[hh][0:64, :], in1=lb_s)
                        nc.sync.dma_start(out=yT_b[64:128, p4, :], in_=tmp)

            nk = 4 * qt + 4
            for p4 in range(4):
                rq = 2 * p4 + 2      # qk groups up to and incl this pair's K,Q

                def rv_of(kr):
                    return max(0, kr - 4 * qt + 1)

                for kr in range(nk):
                    def item(p4=p4, kr=kr):
                        s_group(p4, kr)
                        if kr > 0:
                            o_group(p4, kr - 1)
                    items.append((item, rq, rv_of(kr - 1)))
                items.append(
                    (lambda p4=p4: o_group(p4, nk - 1), rq, 4)
                )
                items.append((lambda p4=p4: norm(p4), rq, 4))
            return items

        def outproj_items(qt):
            yT_b = yT_tiles[qt]
            items = []

            def out_group(st, half):
                ts_ = 4 * qt + st
                ps = p_ps.tile([128, 512], F32, tag="p1", name="ps_out")
                for cc in range(4):
                    nc.tensor.matmul(
                        ps,
                        lhsT=yT_b[:, cc, 128 * st:128 * st + 128],
                        rhs=wp_s[:, cc, 512 * half:512 * half + 512],
                        start=(cc == 0),
                        stop=(cc == 3),
                    )
                ot = p_po.tile([128, 512], BF16, tag="ot", name="ot")
                nc.vector.tensor_copy(out=ot, in_=ps)
                nc.sync.dma_start(
                    out=d_out[128 * ts_:128 * ts_ + 128, 512 * half:512 * half + 512],
                    in_=ot,
                )

            for st in range(4):
                for half in range(2):
                    items.append(lambda st=st, half=half: out_group(st, half))
            return items

        # Same-slot pipeline: proj(qt) groups feed attention(qt) with dep-aware
        # merge; outproj(qt-1) groups are sprinkled through the slot.
        for qt in range(4):
            qk_items, v_items = proj_items(qt)
            b_items = attn_items(qt)
            o_items = outproj_items(qt - 1) if qt >= 1 else []
            ia = iv = io = 0
            if qt == 0:
                # all of slot 0's attention is diagonal (needs V): run the qk
                # matmuls first so the PE isn't stalled on the wv DMAs
                while ia < len(qk_items):
                    qk_items[ia](); ia += 1
            nb = len(b_items)
            for k, (fn, rq, rv) in enumerate(b_items):
                while ia < rq:
                    qk_items[ia](); ia += 1
                while iv < rv:
                    v_items[iv](); iv += 1
                # sprinkle leftovers proportionally to attention progress
                while io < len(o_items) * (k + 1) // nb:
                    o_items[io](); io += 1
                target_a = min(len(qk_items), 2 + (len(qk_items) - 2) * (k + 1) // nb)
                while ia < target_a:
                    qk_items[ia](); ia += 1
                target_v = min(len(v_items), 4 * (k + 1) // max(1, nb - 4))
                while iv < target_v:
                    v_items[iv](); iv += 1
                fn()
            while ia < len(qk_items):
                qk_items[ia](); ia += 1
            while iv < len(v_items):
                v_items[iv](); iv += 1
            while io < len(o_items):
                o_items[io](); io += 1
        for f in outproj_items(3):
            f()


def make_core_inputs(x, W_attn, b_attn, W_proj):
    f = np.float32
    mask = np.where(np.triu(np.ones((128, 128), bool)), 0.0, -30000.0).astype(NPBF16)
    ident = np.eye(128, dtype=f).astype(NPBF16)
    in_maps = []
    for c in range(N_CORES):
        b, g = divmod(c, 2)
        hs = range(HL * g, HL * g + HL)
        xT = np.ascontiguousarray(x[b].T).astype(NPBF16)
        wq = np.concatenate([W_attn[:, h * HD:h * HD + HD] for h in hs], axis=1)
        wk = np.concatenate([W_attn[:, C + h * HD:C + h * HD + HD] for h in hs], axis=1)
        wqk = np.ascontiguousarray(np.concatenate([wq, wk], axis=1)).astype(NPBF16)
        bq = np.concatenate([b_attn[h * HD:h * HD + HD] for h in hs])
        bk = np.concatenate([b_attn[C + h * HD:C + h * HD + HD] for h in hs])
        bqk = np.ascontiguousarray(np.concatenate([bq, bk]))
        wv = np.ascontiguousarray(
            np.concatenate(
                [W_attn[:, 2 * C + h * HD:2 * C + h * HD + HD] for h in hs], axis=1
            )
        ).astype(NPBF16)
        bv = np.ascontiguousarray(
            np.concatenate([b_attn[2 * C + h * HD:2 * C + h * HD + HD] for h in hs])
        )
        wp = np.ascontiguousarray(
            np.concatenate([W_proj[h * HD:h * HD + HD, :] for h in hs], axis=0)
        ).astype(NPBF16)
        in_maps.append(
            {"xT": xT, "wqk": wqk, "wv": wv, "bqk": bqk, "bv": bv, "wp": wp,
             "mask": mask, "ident": ident}
        )
    return in_maps


def kernel(**inputs):
    global LAST_RESULT, _CACHED
    f = np.float32
    x = np.asarray(inputs["x"], f)
    W_attn = np.asarray(inputs["W_attn"], f)
    b_attn = np.asarray(inputs["b_attn"], f)
    W_proj = np.asarray(inputs["W_proj"], f)
    b_proj = np.asarray(inputs["b_proj"], f)

    if _CACHED is None:
        _CACHED = build_kernel()
    nc = _CACHED
    in_maps = make_core_inputs(x, W_attn, b_attn, W_proj)
    res = run_bass_kernel_spmd(nc, in_maps, core_ids=list(range(N_CORES)))
    LAST_RESULT = res
    y = np.empty((B, T, C), f)
    for b in range(B):
        y[b] = (res.results[2 * b]["out"].astype(f)
                + res.results[2 * b + 1]["out"].astype(f) + b_proj)
    return y
